# revision 23
# baseline (speedup 1.0000x reference)
"""AAMSoftmax (norm-free) loss head on 8 Trainium2 NeuronCores.

Math (reference):
    norm    = ||x_b||                                  [B, 1]
    xn      = x / max(norm, eps); wn = W / max(||W_row||, eps)
    cosine  = xn @ wn.T                                [B, OUT]
    phi     = cos(theta + m) = cosine*cos(m) - sine*sin(m)
    out     = norm * where(onehot(label) & cosine > 0, phi, cosine)
    returns (out, wn)

Key identity: norm * cosine == x @ wn.T exactly (norm >> eps), so the big
[B, OUT] tensor is a single matmul of the *unnormalized* x against the
row-normalized weights; the margin substitution only touches B entries
(one per row, at column label_b). Each core computes its 1/8 slab of
out/wn plus the per-row substituted value v_b = norm_b * (cl>0 ? phi : cl)
(cl = cosine at the label column, computed from host-gathered weight rows).
The host assembles slabs and writes v at the B label positions.

Sharding: W rows (out_features) split 8 ways, 8000 rows/core padded to
8192; x/wlab/m replicated.
"""

import functools

import numpy as np

B, IN, OUT = 1024, 512, 64000
NCORES = 8
SLAB = OUT // NCORES  # 8000
PAD = 8192            # per-core weight rows, padded for uniform 128/512 tiling
EPS = 1e-12
HALF_PI = 1.5707963267948966

MM = "bf16"    # matmul operand mode: f32 | f32r | bf16
STORE = "bf16"  # out/wn DRAM dtype: f32 | bf16
INDT = "f32"   # x/w/wlab DRAM dtype: f32 | bf16
XBAR = False   # build wn^T via DMA xbar transpose (needs mm=store=bf16)


@functools.lru_cache(maxsize=None)
def _build(batch=B, in_features=IN, pad=PAD, reps=1, mm=MM, store=STORE, indt=INDT,
           xbar=XBAR, part="all"):
    import concourse.bass as bass
    import concourse.mybir as mybir
    import concourse.tile as tile
    from concourse import bacc
    from concourse.bass import ds, ts
    from concourse.masks import make_identity
    from contextlib import ExitStack

    f32 = mybir.dt.float32
    bf16 = mybir.dt.bfloat16
    f32r = mybir.dt.float32r
    Sin = mybir.ActivationFunctionType.Sin
    Copy = mybir.ActivationFunctionType.Copy
    mult = mybir.AluOpType.mult
    add = mybir.AluOpType.add
    is_gt = mybir.AluOpType.is_gt

    mmdt = {"f32": f32, "f32r": f32r, "bf16": bf16}[mm]  # matmul operand tiles
    stdt = {"f32": f32, "bf16": bf16}[store]             # out/wn storage
    idt = {"f32": f32, "bf16": bf16}[indt]               # x/w/wlab input dtype
    if xbar:
        assert mmdt is bf16 and stdt is bf16, "xbar transpose needs 2-byte dtypes"

    KT = in_features // 128   # contraction chunks
    NBT = batch // 128        # batch tiles
    NOG = pad // 512          # out-feature groups of 512

    nc = bacc.Bacc("TRN2", target_bir_lowering=False, debug=False)

    x_ext = nc.dram_tensor("x", [batch, in_features], idt, kind="ExternalInput")
    w_ext = nc.dram_tensor("w", [pad, in_features], idt, kind="ExternalInput")
    wl_ext = nc.dram_tensor("wlab", [batch, in_features], idt, kind="ExternalInput")
    m_ext = nc.dram_tensor("m", [1, 1], f32, kind="ExternalInput")
    out_ext = nc.dram_tensor("out", [batch, pad], stdt, kind="ExternalOutput")
    wn_ext = nc.dram_tensor("wn", [pad, in_features], stdt, kind="ExternalOutput")
    v_ext = nc.dram_tensor("v", [batch, 1], f32, kind="ExternalOutput")
    trig_dram = nc.dram_tensor("trig_scratch", [1, 2], f32)

    # batched views: partition-major tilings for single-DMA transfers
    x_v = x_ext[:, :].rearrange("(t p) k -> p t k", p=128)       # [128,NBT,IN]
    wl_v = wl_ext[:, :].rearrange("(t p) k -> p t k", p=128)
    v_v = v_ext[:, :].rearrange("(t p) o -> p (t o)", p=128)     # [128,NBT]

    with ExitStack() as ctx:
        tc = ctx.enter_context(tile.TileContext(nc))
        singles = ctx.enter_context(tc.tile_pool(name="singles", bufs=1))
        xin = ctx.enter_context(tc.tile_pool(name="xin", bufs=1))
        small = ctx.enter_context(tc.tile_pool(name="small", bufs=8))
        wpool = ctx.enter_context(tc.tile_pool(name="wpool", bufs=4))
        wnpool = ctx.enter_context(tc.tile_pool(name="wnpool", bufs=4))
        wntp = ctx.enter_context(tc.tile_pool(name="wntp", bufs=4))
        outp = ctx.enter_context(tc.tile_pool(name="outp", bufs=4))
        psum_t = ctx.enter_context(tc.tile_pool(name="psum_t", bufs=3, space="PSUM"))
        psum_mm = ctx.enter_context(tc.tile_pool(name="psum_mm", bufs=5, space="PSUM"))

        def emit_body():
            identity = singles.tile([128, 128], idt)
            make_identity(nc, identity)
            if stdt is not idt:
                identity_st = singles.tile([128, 128], stdt)
                make_identity(nc, identity_st)
            else:
                identity_st = identity

            # cos(m), -sin(m) -> broadcast to all partitions via a DRAM bounce
            m_sb = singles.tile([1, 1], f32)
            nc.sync.dma_start(out=m_sb, in_=m_ext[:, :])
            trig = singles.tile([1, 2], f32)
            halfpi = singles.tile([1, 1], f32)
            nc.vector.memset(halfpi, HALF_PI)
            nc.scalar.activation(trig[:, 0:1], m_sb, Sin, scale=-1.0)    # -sin(m)
            nc.scalar.activation(trig[:, 1:2], m_sb, Sin, bias=halfpi)   # cos(m)
            nc.sync.dma_start(out=trig_dram[:, :], in_=trig)
            trig_bc = singles.tile([128, 2], f32)
            nc.sync.dma_start(out=trig_bc, in_=trig_dram[:, :].to_broadcast([128, 2]))
            negsinm = trig_bc[:, 0:1]
            cosm = trig_bc[:, 1:2]

            # x^T, kept resident: [128, KT, batch]
            xT = singles.tile([128, KT, batch], mmdt)

            xall = xin.tile([128, NBT, in_features], idt)
            nc.sync.dma_start(out=xall, in_=x_v)
            wlall = xin.tile([128, NBT, in_features], idt)
            nc.sync.dma_start(out=wlall, in_=wl_v)
            vstage = xin.tile([128, NBT], f32, tag="vstage")

            for bt in range(NBT):
                xt = xall[:, bt, :]
                wl = wlall[:, bt, :]
                scr = xin.tile([128, in_features], idt, tag="scr", bufs=3)
                ssx = small.tile([128, 1], f32)
                nc.vector.scalar_tensor_tensor(
                    out=scr, in0=xt, scalar=1.0, in1=xt, op0=mult, op1=mult,
                    accum_out=ssx)
                norm = small.tile([128, 1], f32)
                nc.scalar.sqrt(norm, ssx)
                ssw = small.tile([128, 1], f32)
                nc.vector.scalar_tensor_tensor(
                    out=scr, in0=wl, scalar=1.0, in1=wl, op0=mult, op1=mult,
                    accum_out=ssw)
                wnorm = small.tile([128, 1], f32)
                nc.scalar.sqrt(wnorm, ssw)
                dotv = small.tile([128, 1], f32)
                nc.vector.scalar_tensor_tensor(
                    out=scr, in0=xt, scalar=1.0, in1=wl, op0=mult, op1=mult,
                    accum_out=dotv)

                den = small.tile([128, 1], f32)
                normc = small.tile([128, 1], f32)
                nc.vector.tensor_scalar_max(normc, norm, EPS)
                wnormc = small.tile([128, 1], f32)
                nc.vector.tensor_scalar_max(wnormc, wnorm, EPS)
                nc.vector.tensor_mul(den, normc, wnormc)
                rden = small.tile([128, 1], f32)
                nc.vector.reciprocal(rden, den)
                cl = small.tile([128, 1], f32)
                nc.vector.tensor_mul(cl, dotv, rden)

                mask = small.tile([128, 1], f32)
                nc.vector.tensor_scalar(
                    out=mask, in0=cl, scalar1=0.0, scalar2=None, op0=is_gt)
                sine = small.tile([128, 1], f32)
                nc.vector.tensor_mul(sine, cl, cl)
                nc.vector.tensor_scalar(
                    out=sine, in0=sine, scalar1=-1.0, scalar2=1.0, op0=mult, op1=add)
                nc.vector.tensor_scalar_max(sine, sine, 0.0)
                nc.scalar.sqrt(sine, sine)
                phi = small.tile([128, 1], f32)
                nc.vector.tensor_scalar_mul(phi, cl, cosm)
                nc.vector.scalar_tensor_tensor(
                    out=phi, in0=sine, scalar=negsinm, in1=phi, op0=mult, op1=add)
                dv = small.tile([128, 1], f32)
                nc.vector.tensor_sub(dv, phi, cl)
                vv = small.tile([128, 1], f32)
                nc.vector.scalar_tensor_tensor(
                    out=vv, in0=dv, scalar=mask, in1=cl, op0=mult, op1=add)
                nc.vector.tensor_mul(vstage[:, bt:bt + 1], vv, norm)

                pst = psum_t.tile([128, KT, 128], idt, tag="pst")
                for k in range(KT):
                    nc.tensor.transpose(pst[:, k, :], xt[:, ts(k, 128)], identity)
                nc.scalar.copy(out=xT[:, :, ts(bt, 128)], in_=pst)
            nc.sync.dma_start(out=v_v, in_=vstage)

            if part == "mm":
                # timing probe: dense matmul stream against one static wnt
                wnt0 = singles.tile([128, KT, 512], mmdt)
                nc.gpsimd.memset(wnt0, 0.0)
                for og in range(NOG):
                    ostage = outp.tile([128, NBT, 512], stdt)
                    for bt in range(NBT):
                        ps = psum_mm.tile([128, 512], f32)
                        for k in range(KT):
                            nc.tensor.matmul(
                                ps, lhsT=xT[:, k, ts(bt, 128)], rhs=wnt0[:, k, :],
                                start=(k == 0), stop=(k == KT - 1))
                        if bt % 2 == 0:
                            nc.scalar.copy(out=ostage[:, bt, :], in_=ps)
                        else:
                            nc.vector.tensor_copy(out=ostage[:, bt, :], in_=ps)
                    nc.sync.dma_start(
                        out=out_ext[:, ds(og * 512, 512)].rearrange(
                            "(t p) c -> p t c", p=128),
                        in_=ostage)
                return

            for og in range(NOG):
                w4 = wpool.tile([128, 4, in_features], idt)
                nc.sync.dma_start(
                    out=w4,
                    in_=w_ext[ds(og * 512, 512), :].rearrange(
                        "(j p) k -> p j k", p=128))
                wn4 = wnpool.tile([128, 4, in_features], stdt)
                wnt = wntp.tile([128, KT, 512], mmdt)
                for j in range(4):
                    wscr = wpool.tile([128, in_features], idt, tag="wscr")
                    ssw2 = small.tile([128, 1], f32, tag="ssw2")
                    nc.vector.scalar_tensor_tensor(
                        out=wscr, in0=w4[:, j, :], scalar=1.0, in1=w4[:, j, :],
                        op0=mult, op1=mult, accum_out=ssw2)
                    wn_norm = small.tile([128, 1], f32, tag="wn_norm")
                    nc.scalar.sqrt(wn_norm, ssw2)
                    nc.vector.tensor_scalar_max(wn_norm, wn_norm, EPS)
                    winv = small.tile([128, 1], f32, tag="winv")
                    nc.vector.reciprocal(winv, wn_norm)
                    if xbar:
                        nc.gpsimd.tensor_scalar_mul(wn4[:, j, :], w4[:, j, :], winv)
                        nc.sync.dma_start_transpose(
                            out=wnt[:, :, ts(j, 128)], in_=wn4[:, j, :])
                    else:
                        nc.gpsimd.tensor_scalar_mul(wn4[:, j, :], w4[:, j, :], winv)
                        pst2 = psum_t.tile([128, KT, 128], stdt, tag="pst")
                        for k in range(KT):
                            nc.tensor.transpose(
                                pst2[:, k, :], wn4[:, j, ts(k, 128)], identity_st)
                        if j % 2 == 0:
                            nc.scalar.copy(out=wnt[:, :, ts(j, 128)], in_=pst2)
                        else:
                            nc.vector.tensor_copy(out=wnt[:, :, ts(j, 128)], in_=pst2)
                nc.sync.dma_start(
                    out=wn_ext[ds(og * 512, 512), :].rearrange(
                        "(j p) k -> p j k", p=128),
                    in_=wn4)
                if part == "prep":
                    continue

                ostage = outp.tile([128, NBT, 512], stdt)
                for bt in range(NBT):
                    ps = psum_mm.tile([128, 512], f32)
                    for k in range(KT):
                        nc.tensor.matmul(
                            ps, lhsT=xT[:, k, ts(bt, 128)], rhs=wnt[:, k, :],
                            start=(k == 0), stop=(k == KT - 1))
                    if bt % 4 == 1:
                        nc.vector.tensor_copy(out=ostage[:, bt, :], in_=ps)
                    else:
                        nc.scalar.copy(out=ostage[:, bt, :], in_=ps)
                nc.sync.dma_start(
                    out=out_ext[:, ds(og * 512, 512)].rearrange(
                        "(t p) c -> p t c", p=128),
                    in_=ostage)

        if reps == 1:
            emit_body()
        else:
            with tc.For_i(0, reps, 1):
                emit_body()

    nc.compile()
    return nc


def _run(nc, in_maps, trace=False):
    from concourse.bass_utils import run_bass_kernel_spmd

    return run_bass_kernel_spmd(
        nc, in_maps, core_ids=list(range(len(in_maps))), trace=trace)


def _make_in_maps(x, label, weight, m, indt=INDT):
    if indt == "bf16":
        import ml_dtypes

        cast = lambda a: a.astype(ml_dtypes.bfloat16)
    else:
        cast = lambda a: a
    wlab = np.ascontiguousarray(weight[label])  # [B, IN] host gather
    x = cast(x)
    wlab = cast(wlab)
    in_maps = []
    for c in range(NCORES):
        wpad = np.zeros((PAD, IN), np.float32)
        wpad[:SLAB] = weight[c * SLAB:(c + 1) * SLAB]
        in_maps.append({"x": x, "w": cast(wpad), "wlab": wlab, "m": m})
    return in_maps


def _assemble(res, label):
    out = np.concatenate(
        [np.asarray(r["out"][:, :SLAB], dtype=np.float32) for r in res], axis=1)
    wn = np.concatenate(
        [np.asarray(r["wn"][:SLAB], dtype=np.float32) for r in res], axis=0)
    out[np.arange(B), label] = res[0]["v"][:, 0]
    return out, wn


def kernel(**inputs):
    x = np.asarray(inputs["x"], dtype=np.float32)
    label = np.asarray(inputs["label"]).astype(np.int64)
    weight = np.asarray(inputs["weight"], dtype=np.float32)
    m = np.asarray(inputs["m"], dtype=np.float32).reshape(1, 1)

    nc = _build()
    res = _run(nc, _make_in_maps(x, label, weight, m)).results
    return _assemble(res, label)


# revision 24
# speedup vs baseline: 2.4049x; 2.4049x over previous
"""AAMSoftmax (norm-free) loss head on 8 Trainium2 NeuronCores.

Math (reference):
    norm    = ||x_b||                                  [B, 1]
    xn      = x / max(norm, eps); wn = W / max(||W_row||, eps)
    cosine  = xn @ wn.T                                [B, OUT]
    phi     = cos(theta + m) = cosine*cos(m) - sine*sin(m)
    out     = norm * where(onehot(label) & cosine > 0, phi, cosine)
    returns (out, wn)

Key identity: norm * cosine == x @ wn.T exactly (norm >> eps), so the big
[B, OUT] tensor is a single matmul of the *unnormalized* x against the
row-normalized weights; the margin substitution only touches B entries
(one per row, at column label_b). Each core computes its 1/8 slab of
out/wn plus the per-row substituted value v_b = norm_b * (cl>0 ? phi : cl)
(cl = cosine at the label column, computed from host-gathered weight rows).
The host assembles slabs and writes v at the B label positions.

Sharding: W rows (out_features) split 8 ways, 8000 rows/core padded to
8192; x/wlab/m replicated.
"""

import functools

import numpy as np

B, IN, OUT = 1024, 512, 64000
NCORES = 8
SLAB = OUT // NCORES  # 8000
PAD = 8192            # per-core weight rows, padded for uniform 128/512 tiling
EPS = 1e-12
HALF_PI = 1.5707963267948966

MM = "bf16"    # matmul operand mode: f32 | f32r | bf16
STORE = "bf16"  # out/wn DRAM dtype: f32 | bf16
INDT = "f32"   # x/w/wlab DRAM dtype: f32 | bf16
XBAR = False   # build wn^T via DMA xbar transpose (needs mm=store=bf16)


@functools.lru_cache(maxsize=None)
def _build(batch=B, in_features=IN, pad=PAD, reps=1, mm=MM, store=STORE, indt=INDT,
           xbar=XBAR, part="all"):
    import concourse.bass as bass
    import concourse.mybir as mybir
    import concourse.tile as tile
    from concourse import bacc
    from concourse.bass import ds, ts
    from concourse.masks import make_identity
    from contextlib import ExitStack

    f32 = mybir.dt.float32
    bf16 = mybir.dt.bfloat16
    f32r = mybir.dt.float32r
    Sin = mybir.ActivationFunctionType.Sin
    Copy = mybir.ActivationFunctionType.Copy
    mult = mybir.AluOpType.mult
    add = mybir.AluOpType.add
    is_gt = mybir.AluOpType.is_gt

    mmdt = {"f32": f32, "f32r": f32r, "bf16": bf16}[mm]  # matmul operand tiles
    stdt = {"f32": f32, "bf16": bf16}[store]             # out/wn storage
    idt = {"f32": f32, "bf16": bf16}[indt]               # x/w/wlab input dtype
    if xbar:
        assert mmdt is bf16 and stdt is bf16, "xbar transpose needs 2-byte dtypes"

    KT = in_features // 128   # contraction chunks
    NBT = batch // 128        # batch tiles
    NOG = pad // 512          # out-feature groups of 512

    nc = bacc.Bacc("TRN2", target_bir_lowering=False, debug=False)

    x_ext = nc.dram_tensor("x", [batch, in_features], idt, kind="ExternalInput")
    w_ext = nc.dram_tensor("w", [pad, in_features], idt, kind="ExternalInput")
    wl_ext = nc.dram_tensor("wlab", [batch, in_features], idt, kind="ExternalInput")
    m_ext = nc.dram_tensor("m", [1, 1], f32, kind="ExternalInput")
    out_ext = nc.dram_tensor("out", [batch, pad], stdt, kind="ExternalOutput")
    wn_ext = nc.dram_tensor("wn", [pad, in_features], stdt, kind="ExternalOutput")
    v_ext = nc.dram_tensor("v", [batch, 1], f32, kind="ExternalOutput")
    trig_dram = nc.dram_tensor("trig_scratch", [1, 2], f32)

    # batched views: partition-major tilings for single-DMA transfers
    x_v = x_ext[:, :].rearrange("(t p) k -> p t k", p=128)       # [128,NBT,IN]
    wl_v = wl_ext[:, :].rearrange("(t p) k -> p t k", p=128)
    v_v = v_ext[:, :].rearrange("(t p) o -> p (t o)", p=128)     # [128,NBT]

    with ExitStack() as ctx:
        tc = ctx.enter_context(tile.TileContext(nc))
        singles = ctx.enter_context(tc.tile_pool(name="singles", bufs=1))
        xin = ctx.enter_context(tc.tile_pool(name="xin", bufs=1))
        small = ctx.enter_context(tc.tile_pool(name="small", bufs=8))
        wpool = ctx.enter_context(tc.tile_pool(name="wpool", bufs=4))
        wnpool = ctx.enter_context(tc.tile_pool(name="wnpool", bufs=4))
        wntp = ctx.enter_context(tc.tile_pool(name="wntp", bufs=4))
        outp = ctx.enter_context(tc.tile_pool(name="outp", bufs=4))
        psum_t = ctx.enter_context(tc.tile_pool(name="psum_t", bufs=3, space="PSUM"))
        psum_mm = ctx.enter_context(tc.tile_pool(name="psum_mm", bufs=5, space="PSUM"))

        def emit_body():
            identity = singles.tile([128, 128], idt)
            make_identity(nc, identity)
            if stdt is not idt:
                identity_st = singles.tile([128, 128], stdt)
                make_identity(nc, identity_st)
            else:
                identity_st = identity

            # cos(m), -sin(m) -> broadcast to all partitions via a DRAM bounce
            m_sb = singles.tile([1, 1], f32)
            nc.sync.dma_start(out=m_sb, in_=m_ext[:, :])
            trig = singles.tile([1, 2], f32)
            halfpi = singles.tile([1, 1], f32)
            nc.vector.memset(halfpi, HALF_PI)
            nc.scalar.activation(trig[:, 0:1], m_sb, Sin, scale=-1.0)    # -sin(m)
            nc.scalar.activation(trig[:, 1:2], m_sb, Sin, bias=halfpi)   # cos(m)
            nc.sync.dma_start(out=trig_dram[:, :], in_=trig)
            trig_bc = singles.tile([128, 2], f32)
            nc.sync.dma_start(out=trig_bc, in_=trig_dram[:, :].to_broadcast([128, 2]))
            negsinm = trig_bc[:, 0:1]
            cosm = trig_bc[:, 1:2]

            # x^T, kept resident: [128, KT, batch]
            xT = singles.tile([128, KT, batch], mmdt)

            xall = xin.tile([128, NBT, in_features], idt)
            nc.sync.dma_start(out=xall, in_=x_v)
            wlall = xin.tile([128, NBT, in_features], idt)
            nc.sync.dma_start(out=wlall, in_=wl_v)
            vstage = xin.tile([128, NBT], f32, tag="vstage")

            for bt in range(NBT):
                xt = xall[:, bt, :]
                wl = wlall[:, bt, :]
                scr = xin.tile([128, in_features], idt, tag="scr", bufs=3)
                ssx = small.tile([128, 1], f32)
                nc.vector.scalar_tensor_tensor(
                    out=scr, in0=xt, scalar=1.0, in1=xt, op0=mult, op1=mult,
                    accum_out=ssx)
                norm = small.tile([128, 1], f32)
                nc.scalar.sqrt(norm, ssx)
                ssw = small.tile([128, 1], f32)
                nc.vector.scalar_tensor_tensor(
                    out=scr, in0=wl, scalar=1.0, in1=wl, op0=mult, op1=mult,
                    accum_out=ssw)
                wnorm = small.tile([128, 1], f32)
                nc.scalar.sqrt(wnorm, ssw)
                dotv = small.tile([128, 1], f32)
                nc.vector.scalar_tensor_tensor(
                    out=scr, in0=xt, scalar=1.0, in1=wl, op0=mult, op1=mult,
                    accum_out=dotv)

                den = small.tile([128, 1], f32)
                normc = small.tile([128, 1], f32)
                nc.vector.tensor_scalar_max(normc, norm, EPS)
                wnormc = small.tile([128, 1], f32)
                nc.vector.tensor_scalar_max(wnormc, wnorm, EPS)
                nc.vector.tensor_mul(den, normc, wnormc)
                rden = small.tile([128, 1], f32)
                nc.vector.reciprocal(rden, den)
                cl = small.tile([128, 1], f32)
                nc.vector.tensor_mul(cl, dotv, rden)

                mask = small.tile([128, 1], f32)
                nc.vector.tensor_scalar(
                    out=mask, in0=cl, scalar1=0.0, scalar2=None, op0=is_gt)
                sine = small.tile([128, 1], f32)
                nc.vector.tensor_mul(sine, cl, cl)
                nc.vector.tensor_scalar(
                    out=sine, in0=sine, scalar1=-1.0, scalar2=1.0, op0=mult, op1=add)
                nc.vector.tensor_scalar_max(sine, sine, 0.0)
                nc.scalar.sqrt(sine, sine)
                phi = small.tile([128, 1], f32)
                nc.vector.tensor_scalar_mul(phi, cl, cosm)
                nc.vector.scalar_tensor_tensor(
                    out=phi, in0=sine, scalar=negsinm, in1=phi, op0=mult, op1=add)
                dv = small.tile([128, 1], f32)
                nc.vector.tensor_sub(dv, phi, cl)
                vv = small.tile([128, 1], f32)
                nc.vector.scalar_tensor_tensor(
                    out=vv, in0=dv, scalar=mask, in1=cl, op0=mult, op1=add)
                nc.vector.tensor_mul(vstage[:, bt:bt + 1], vv, norm)

                pst = psum_t.tile([128, KT, 128], idt, tag="pst")
                for k in range(KT):
                    nc.tensor.transpose(pst[:, k, :], xt[:, ts(k, 128)], identity)
                nc.scalar.copy(out=xT[:, :, ts(bt, 128)], in_=pst)
            nc.sync.dma_start(out=v_v, in_=vstage)

            if part == "mm":
                # timing probe: dense matmul stream against one static wnt
                wnt0 = singles.tile([128, KT, 512], mmdt)
                nc.gpsimd.memset(wnt0, 0.0)
                for og in range(NOG):
                    ostage = outp.tile([128, NBT, 512], stdt)
                    for bt in range(NBT):
                        ps = psum_mm.tile([128, 512], f32)
                        for k in range(KT):
                            nc.tensor.matmul(
                                ps, lhsT=xT[:, k, ts(bt, 128)], rhs=wnt0[:, k, :],
                                start=(k == 0), stop=(k == KT - 1))
                        if bt % 2 == 0:
                            nc.scalar.copy(out=ostage[:, bt, :], in_=ps)
                        else:
                            nc.vector.tensor_copy(out=ostage[:, bt, :], in_=ps)
                    nc.sync.dma_start(
                        out=out_ext[:, ds(og * 512, 512)].rearrange(
                            "(t p) c -> p t c", p=128),
                        in_=ostage)
                return

            for og in range(NOG):
                w4 = wpool.tile([128, 4, in_features], idt)
                nc.sync.dma_start(
                    out=w4,
                    in_=w_ext[ds(og * 512, 512), :].rearrange(
                        "(j p) k -> p j k", p=128))
                wn4 = wnpool.tile([128, 4, in_features], stdt)
                wnt = wntp.tile([128, KT, 512], mmdt)
                for j in range(4):
                    wscr = wpool.tile([128, in_features], idt, tag="wscr")
                    ssw2 = small.tile([128, 1], f32, tag="ssw2")
                    nc.vector.scalar_tensor_tensor(
                        out=wscr, in0=w4[:, j, :], scalar=1.0, in1=w4[:, j, :],
                        op0=mult, op1=mult, accum_out=ssw2)
                    wn_norm = small.tile([128, 1], f32, tag="wn_norm")
                    nc.scalar.sqrt(wn_norm, ssw2)
                    nc.vector.tensor_scalar_max(wn_norm, wn_norm, EPS)
                    winv = small.tile([128, 1], f32, tag="winv")
                    nc.vector.reciprocal(winv, wn_norm)
                    if xbar:
                        nc.gpsimd.tensor_scalar_mul(wn4[:, j, :], w4[:, j, :], winv)
                        nc.sync.dma_start_transpose(
                            out=wnt[:, :, ts(j, 128)], in_=wn4[:, j, :])
                    else:
                        if j % 2 == 0:
                            nc.vector.tensor_scalar_mul(
                                wn4[:, j, :], w4[:, j, :], winv)
                        else:
                            nc.scalar.mul(wn4[:, j, :], w4[:, j, :], winv)
                        pst2 = psum_t.tile([128, KT, 128], stdt, tag="pst")
                        for k in range(KT):
                            nc.tensor.transpose(
                                pst2[:, k, :], wn4[:, j, ts(k, 128)], identity_st)
                        if j % 2 == 0:
                            nc.scalar.copy(out=wnt[:, :, ts(j, 128)], in_=pst2)
                        else:
                            nc.vector.tensor_copy(out=wnt[:, :, ts(j, 128)], in_=pst2)
                nc.sync.dma_start(
                    out=wn_ext[ds(og * 512, 512), :].rearrange(
                        "(j p) k -> p j k", p=128),
                    in_=wn4)
                if part == "prep":
                    continue

                ostage = outp.tile([128, NBT, 512], stdt)
                for bt in range(NBT):
                    ps = psum_mm.tile([128, 512], f32)
                    for k in range(KT):
                        nc.tensor.matmul(
                            ps, lhsT=xT[:, k, ts(bt, 128)], rhs=wnt[:, k, :],
                            start=(k == 0), stop=(k == KT - 1))
                    if bt % 4 == 1:
                        nc.vector.tensor_copy(out=ostage[:, bt, :], in_=ps)
                    else:
                        nc.scalar.copy(out=ostage[:, bt, :], in_=ps)
                nc.sync.dma_start(
                    out=out_ext[:, ds(og * 512, 512)].rearrange(
                        "(t p) c -> p t c", p=128),
                    in_=ostage)

        if reps == 1:
            emit_body()
        else:
            with tc.For_i(0, reps, 1):
                emit_body()

    nc.compile()
    return nc


def _run(nc, in_maps, trace=False):
    from concourse.bass_utils import run_bass_kernel_spmd

    return run_bass_kernel_spmd(
        nc, in_maps, core_ids=list(range(len(in_maps))), trace=trace)


def _make_in_maps(x, label, weight, m, indt=INDT):
    if indt == "bf16":
        import ml_dtypes

        cast = lambda a: a.astype(ml_dtypes.bfloat16)
    else:
        cast = lambda a: a
    wlab = np.ascontiguousarray(weight[label])  # [B, IN] host gather
    x = cast(x)
    wlab = cast(wlab)
    in_maps = []
    for c in range(NCORES):
        wpad = np.zeros((PAD, IN), np.float32)
        wpad[:SLAB] = weight[c * SLAB:(c + 1) * SLAB]
        in_maps.append({"x": x, "w": cast(wpad), "wlab": wlab, "m": m})
    return in_maps


def _assemble(res, label):
    out = np.concatenate(
        [np.asarray(r["out"][:, :SLAB], dtype=np.float32) for r in res], axis=1)
    wn = np.concatenate(
        [np.asarray(r["wn"][:SLAB], dtype=np.float32) for r in res], axis=0)
    out[np.arange(B), label] = res[0]["v"][:, 0]
    return out, wn


def kernel(**inputs):
    x = np.asarray(inputs["x"], dtype=np.float32)
    label = np.asarray(inputs["label"]).astype(np.int64)
    weight = np.asarray(inputs["weight"], dtype=np.float32)
    m = np.asarray(inputs["m"], dtype=np.float32).reshape(1, 1)

    nc = _build()
    res = _run(nc, _make_in_maps(x, label, weight, m)).results
    return _assemble(res, label)


# revision 25
# speedup vs baseline: 2.5509x; 1.0607x over previous
"""AAMSoftmax (norm-free) loss head on 8 Trainium2 NeuronCores.

Math (reference):
    norm    = ||x_b||                                  [B, 1]
    xn      = x / max(norm, eps); wn = W / max(||W_row||, eps)
    cosine  = xn @ wn.T                                [B, OUT]
    phi     = cos(theta + m) = cosine*cos(m) - sine*sin(m)
    out     = norm * where(onehot(label) & cosine > 0, phi, cosine)
    returns (out, wn)

Key identity: norm * cosine == x @ wn.T exactly (norm >> eps), so the big
[B, OUT] tensor is a single matmul of the *unnormalized* x against the
row-normalized weights; the margin substitution only touches B entries
(one per row, at column label_b). Each core computes its 1/8 slab of
out/wn plus the per-row substituted value v_b = norm_b * (cl>0 ? phi : cl)
(cl = cosine at the label column, computed from host-gathered weight rows).
The host assembles slabs and writes v at the B label positions.

Sharding: W rows (out_features) split 8 ways, 8000 rows/core padded to
8192; x/wlab/m replicated. All device I/O tensors use partition-major
packed layouts (host packs/unpacks) so every DMA is fully contiguous.
"""

import functools

import numpy as np

B, IN, OUT = 1024, 512, 64000
NCORES = 8
SLAB = OUT // NCORES  # 8000
PAD = 8192            # per-core weight rows, padded for uniform 128/512 tiling
EPS = 1e-12
HALF_PI = 1.5707963267948966

KT = IN // 128    # contraction chunks (4)
NBT = B // 128    # batch tiles (8)
NOG = PAD // 512  # out-feature groups (16)

MM = "bf16"     # matmul operand mode: f32 | f32r | bf16
STORE = "bf16"  # out/wn DRAM dtype: f32 | bf16
INDT = "f32"    # x/w/wlab DRAM dtype: f32 | bf16
XBAR = False    # build wn^T via DMA xbar transpose (needs mm=store=bf16)


@functools.lru_cache(maxsize=None)
def _build(reps=1, mm=MM, store=STORE, indt=INDT, xbar=XBAR, part="all"):
    import concourse.bass as bass
    import concourse.mybir as mybir
    import concourse.tile as tile
    from concourse import bacc
    from concourse.bass import ds, ts
    from concourse.masks import make_identity
    from contextlib import ExitStack

    f32 = mybir.dt.float32
    bf16 = mybir.dt.bfloat16
    f32r = mybir.dt.float32r
    Sin = mybir.ActivationFunctionType.Sin
    mult = mybir.AluOpType.mult
    add = mybir.AluOpType.add
    is_gt = mybir.AluOpType.is_gt

    mmdt = {"f32": f32, "f32r": f32r, "bf16": bf16}[mm]  # matmul operand tiles
    stdt = {"f32": f32, "bf16": bf16}[store]             # out/wn storage
    idt = {"f32": f32, "bf16": bf16}[indt]               # x/w/wlab input dtype
    if xbar:
        assert mmdt is bf16 and stdt is bf16, "xbar transpose needs 2-byte dtypes"

    nc = bacc.Bacc("TRN2", target_bir_lowering=False, debug=False)

    # packed, partition-major layouts -> every DMA is contiguous
    x_ext = nc.dram_tensor("x", [128, NBT, IN], idt, kind="ExternalInput")
    w_ext = nc.dram_tensor("w", [NOG, 128, 4, IN], idt, kind="ExternalInput")
    wl_ext = nc.dram_tensor("wlab", [128, NBT, IN], idt, kind="ExternalInput")
    m_ext = nc.dram_tensor("m", [1, 1], f32, kind="ExternalInput")
    out_ext = nc.dram_tensor("out", [NOG, 128, NBT, 512], stdt,
                             kind="ExternalOutput")
    wn_ext = nc.dram_tensor("wn", [NOG, 128, 4, IN], stdt, kind="ExternalOutput")
    v_ext = nc.dram_tensor("v", [128, NBT], f32, kind="ExternalOutput")
    trig_dram = nc.dram_tensor("trig_scratch", [1, 2], f32)

    with ExitStack() as ctx:
        tc = ctx.enter_context(tile.TileContext(nc))
        singles = ctx.enter_context(tc.tile_pool(name="singles", bufs=1))
        xin = ctx.enter_context(tc.tile_pool(name="xin", bufs=1))
        small = ctx.enter_context(tc.tile_pool(name="small", bufs=8))
        wpool = ctx.enter_context(tc.tile_pool(name="wpool", bufs=4))
        wnpool = ctx.enter_context(tc.tile_pool(name="wnpool", bufs=4))
        wntp = ctx.enter_context(tc.tile_pool(name="wntp", bufs=4))
        outp = ctx.enter_context(tc.tile_pool(name="outp", bufs=4))
        psum_t = ctx.enter_context(tc.tile_pool(name="psum_t", bufs=3, space="PSUM"))
        psum_mm = ctx.enter_context(tc.tile_pool(name="psum_mm", bufs=5, space="PSUM"))

        def emit_body():
            identity = singles.tile([128, 128], idt)
            make_identity(nc, identity)
            if stdt is not idt:
                identity_st = singles.tile([128, 128], stdt)
                make_identity(nc, identity_st)
            else:
                identity_st = identity

            # cos(m), -sin(m) -> broadcast to all partitions via a DRAM bounce
            m_sb = singles.tile([1, 1], f32)
            nc.sync.dma_start(out=m_sb, in_=m_ext[:, :])
            trig = singles.tile([1, 2], f32)
            halfpi = singles.tile([1, 1], f32)
            nc.vector.memset(halfpi, HALF_PI)
            nc.scalar.activation(trig[:, 0:1], m_sb, Sin, scale=-1.0)    # -sin(m)
            nc.scalar.activation(trig[:, 1:2], m_sb, Sin, bias=halfpi)   # cos(m)
            nc.sync.dma_start(out=trig_dram[:, :], in_=trig)
            trig_bc = singles.tile([128, 2], f32)
            nc.sync.dma_start(out=trig_bc, in_=trig_dram[:, :].to_broadcast([128, 2]))
            negsinm = trig_bc[:, 0:1]
            cosm = trig_bc[:, 1:2]

            # x^T, kept resident: [128, KT, batch]
            xT = singles.tile([128, KT, B], mmdt)

            xall = xin.tile([128, NBT, IN], idt)
            nc.sync.dma_start(out=xall, in_=x_ext[:, :, :])
            wlall = xin.tile([128, NBT, IN], idt)
            nc.sync.dma_start(out=wlall, in_=wl_ext[:, :, :])
            vstage = xin.tile([128, NBT], f32, tag="vstage")

            for bt in range(NBT):
                xt = xall[:, bt, :]
                wl = wlall[:, bt, :]
                scr = xin.tile([128, IN], idt, tag="scr", bufs=3)
                ssx = small.tile([128, 1], f32)
                nc.vector.scalar_tensor_tensor(
                    out=scr, in0=xt, scalar=1.0, in1=xt, op0=mult, op1=mult,
                    accum_out=ssx)
                norm = small.tile([128, 1], f32)
                nc.scalar.sqrt(norm, ssx)
                ssw = small.tile([128, 1], f32)
                nc.vector.scalar_tensor_tensor(
                    out=scr, in0=wl, scalar=1.0, in1=wl, op0=mult, op1=mult,
                    accum_out=ssw)
                wnorm = small.tile([128, 1], f32)
                nc.scalar.sqrt(wnorm, ssw)
                dotv = small.tile([128, 1], f32)
                nc.vector.scalar_tensor_tensor(
                    out=scr, in0=xt, scalar=1.0, in1=wl, op0=mult, op1=mult,
                    accum_out=dotv)

                den = small.tile([128, 1], f32)
                normc = small.tile([128, 1], f32)
                nc.vector.tensor_scalar_max(normc, norm, EPS)
                wnormc = small.tile([128, 1], f32)
                nc.vector.tensor_scalar_max(wnormc, wnorm, EPS)
                nc.vector.tensor_mul(den, normc, wnormc)
                rden = small.tile([128, 1], f32)
                nc.vector.reciprocal(rden, den)
                cl = small.tile([128, 1], f32)
                nc.vector.tensor_mul(cl, dotv, rden)

                mask = small.tile([128, 1], f32)
                nc.vector.tensor_scalar(
                    out=mask, in0=cl, scalar1=0.0, scalar2=None, op0=is_gt)
                sine = small.tile([128, 1], f32)
                nc.vector.tensor_mul(sine, cl, cl)
                nc.vector.tensor_scalar(
                    out=sine, in0=sine, scalar1=-1.0, scalar2=1.0, op0=mult, op1=add)
                nc.vector.tensor_scalar_max(sine, sine, 0.0)
                nc.scalar.sqrt(sine, sine)
                phi = small.tile([128, 1], f32)
                nc.vector.tensor_scalar_mul(phi, cl, cosm)
                nc.vector.scalar_tensor_tensor(
                    out=phi, in0=sine, scalar=negsinm, in1=phi, op0=mult, op1=add)
                dv = small.tile([128, 1], f32)
                nc.vector.tensor_sub(dv, phi, cl)
                vv = small.tile([128, 1], f32)
                nc.vector.scalar_tensor_tensor(
                    out=vv, in0=dv, scalar=mask, in1=cl, op0=mult, op1=add)
                nc.vector.tensor_mul(vstage[:, bt:bt + 1], vv, norm)

                pst = psum_t.tile([128, KT, 128], idt, tag="pst")
                for k in range(KT):
                    nc.tensor.transpose(pst[:, k, :], xt[:, ts(k, 128)], identity)
                nc.scalar.copy(out=xT[:, :, ts(bt, 128)], in_=pst)
            nc.sync.dma_start(out=v_ext[:, :], in_=vstage)

            if part == "mm":
                # timing probe: dense matmul stream against one static wnt
                wnt0 = singles.tile([128, KT, 512], mmdt)
                nc.gpsimd.memset(wnt0, 0.0)
                for og in range(NOG):
                    ostage = outp.tile([128, NBT, 512], stdt)
                    for bt in range(NBT):
                        ps = psum_mm.tile([128, 512], f32)
                        for k in range(KT):
                            nc.tensor.matmul(
                                ps, lhsT=xT[:, k, ts(bt, 128)], rhs=wnt0[:, k, :],
                                start=(k == 0), stop=(k == KT - 1))
                        if bt % 2 == 0:
                            nc.scalar.copy(out=ostage[:, bt, :], in_=ps)
                        else:
                            nc.vector.tensor_copy(out=ostage[:, bt, :], in_=ps)
                    nc.sync.dma_start(out=out_ext[og], in_=ostage)
                return

            for og in range(NOG):
                w4 = wpool.tile([128, 4, IN], idt)
                nc.sync.dma_start(out=w4, in_=w_ext[og])
                wn4 = wnpool.tile([128, 4, IN], stdt)
                wnt = wntp.tile([128, KT, 512], mmdt)

                ssw4 = small.tile([128, 4], f32, tag="ssw4")
                for j in range(4):
                    wscr = wpool.tile([128, IN], idt, tag="wscr")
                    nc.vector.scalar_tensor_tensor(
                        out=wscr, in0=w4[:, j, :], scalar=1.0, in1=w4[:, j, :],
                        op0=mult, op1=mult, accum_out=ssw4[:, j:j + 1])
                wnn4 = small.tile([128, 4], f32, tag="wnn4")
                nc.scalar.sqrt(wnn4, ssw4)
                nc.vector.tensor_scalar_max(wnn4, wnn4, EPS)
                winv4 = small.tile([128, 4], f32, tag="winv4")
                nc.vector.reciprocal(winv4, wnn4)

                for j in range(4):
                    if xbar:
                        nc.gpsimd.tensor_scalar_mul(
                            wn4[:, j, :], w4[:, j, :], winv4[:, j:j + 1])
                        nc.sync.dma_start_transpose(
                            out=wnt[:, :, ts(j, 128)], in_=wn4[:, j, :])
                    else:
                        if j % 2 == 0:
                            nc.vector.tensor_scalar_mul(
                                wn4[:, j, :], w4[:, j, :], winv4[:, j:j + 1])
                        else:
                            nc.scalar.mul(
                                wn4[:, j, :], w4[:, j, :], winv4[:, j:j + 1])
                        pst2 = psum_t.tile([128, KT, 128], stdt, tag="pst")
                        for k in range(KT):
                            nc.tensor.transpose(
                                pst2[:, k, :], wn4[:, j, ts(k, 128)], identity_st)
                        if j % 2 == 0:
                            nc.scalar.copy(out=wnt[:, :, ts(j, 128)], in_=pst2)
                        else:
                            nc.vector.tensor_copy(out=wnt[:, :, ts(j, 128)], in_=pst2)
                nc.sync.dma_start(out=wn_ext[og], in_=wn4)
                if part == "prep":
                    continue

                ostage = outp.tile([128, NBT, 512], stdt)
                for bt in range(NBT):
                    ps = psum_mm.tile([128, 512], f32)
                    for k in range(KT):
                        nc.tensor.matmul(
                            ps, lhsT=xT[:, k, ts(bt, 128)], rhs=wnt[:, k, :],
                            start=(k == 0), stop=(k == KT - 1))
                    if bt % 2 == 0:
                        nc.scalar.copy(out=ostage[:, bt, :], in_=ps)
                    else:
                        nc.vector.tensor_copy(out=ostage[:, bt, :], in_=ps)
                nc.sync.dma_start(out=out_ext[og], in_=ostage)

        if reps == 1:
            emit_body()
        else:
            with tc.For_i(0, reps, 1):
                emit_body()

    nc.compile()
    return nc


def _run(nc, in_maps, trace=False):
    from concourse.bass_utils import run_bass_kernel_spmd

    return run_bass_kernel_spmd(
        nc, in_maps, core_ids=list(range(len(in_maps))), trace=trace)


def _pack_bt(a):
    """[B, IN] row-major -> [128, NBT, IN] partition-major."""
    return np.ascontiguousarray(a.reshape(NBT, 128, IN).swapaxes(0, 1))


def _make_in_maps(x, label, weight, m, indt=INDT):
    if indt == "bf16":
        import ml_dtypes

        cast = lambda a: a.astype(ml_dtypes.bfloat16)
    else:
        cast = lambda a: np.ascontiguousarray(a, dtype=np.float32)
    wlab = weight[label]  # [B, IN] host gather
    xp = cast(_pack_bt(x))
    wlp = cast(_pack_bt(wlab))
    in_maps = []
    for c in range(NCORES):
        wpad = np.zeros((PAD, IN), np.float32)
        wpad[:SLAB] = weight[c * SLAB:(c + 1) * SLAB]
        # [PAD, IN] -> [NOG, 128, 4, IN]: row o = og*512 + j*128 + p
        wp = wpad.reshape(NOG, 4, 128, IN).swapaxes(1, 2)
        in_maps.append({"x": xp, "w": cast(wp), "wlab": wlp, "m": m})
    return in_maps


def _assemble(res, label):
    outs, wns = [], []
    for r in res:
        od = np.asarray(r["out"], dtype=np.float32)     # [NOG,128,NBT,512]
        outs.append(od.transpose(2, 1, 0, 3).reshape(B, PAD)[:, :SLAB])
        wd = np.asarray(r["wn"], dtype=np.float32)      # [NOG,128,4,IN]
        wns.append(wd.swapaxes(1, 2).reshape(PAD, IN)[:SLAB])
    out = np.concatenate(outs, axis=1)
    wn = np.concatenate(wns, axis=0)
    v = np.asarray(res[0]["v"])                         # [128, NBT]
    out[np.arange(B), label] = v.T.reshape(B)
    return out, wn


def kernel(**inputs):
    x = np.asarray(inputs["x"], dtype=np.float32)
    label = np.asarray(inputs["label"]).astype(np.int64)
    weight = np.asarray(inputs["weight"], dtype=np.float32)
    m = np.asarray(inputs["m"], dtype=np.float32).reshape(1, 1)

    nc = _build()
    res = _run(nc, _make_in_maps(x, label, weight, m)).results
    return _assemble(res, label)


# revision 26
# speedup vs baseline: 2.9278x; 1.1478x over previous
"""AAMSoftmax (norm-free) loss head on 8 Trainium2 NeuronCores.

Math (reference):
    norm    = ||x_b||                                  [B, 1]
    xn      = x / max(norm, eps); wn = W / max(||W_row||, eps)
    cosine  = xn @ wn.T                                [B, OUT]
    phi     = cos(theta + m) = cosine*cos(m) - sine*sin(m)
    out     = norm * where(onehot(label) & cosine > 0, phi, cosine)
    returns (out, wn)

Key identity: norm * cosine == x @ wn.T exactly (norm >> eps), so the big
[B, OUT] tensor is a single matmul of the *unnormalized* x against the
row-normalized weights; the margin substitution only touches B entries
(one per row, at column label_b). Each core computes its 1/8 slab of
out/wn plus the per-row substituted value v_b = norm_b * (cl>0 ? phi : cl)
(cl = cosine at the label column, computed from host-gathered weight rows).
The host assembles slabs and writes v at the B label positions.

Sharding: W rows (out_features) split 8 ways, 8000 rows/core padded to
8192; x/wlab/m replicated. All device I/O tensors use partition-major
packed layouts (host packs/unpacks) so every DMA is fully contiguous.
"""

import functools

import numpy as np

B, IN, OUT = 1024, 512, 64000
NCORES = 8
SLAB = OUT // NCORES  # 8000
PAD = 8192            # per-core weight rows, padded for uniform 128/512 tiling
EPS = 1e-12
HALF_PI = 1.5707963267948966

KT = IN // 128    # contraction chunks (4)
NBT = B // 128    # batch tiles (8)
NOG = PAD // 512  # out-feature groups (16)

MM = "bf16"     # matmul operand mode: f32 | f32r | bf16
STORE = "bf16"  # out/wn DRAM dtype: f32 | bf16
INDT = "f32"    # x/w/wlab DRAM dtype: f32 | bf16
XBAR = False    # build wn^T via DMA xbar transpose (needs mm=store=bf16)


@functools.lru_cache(maxsize=None)
def _build(reps=1, mm=MM, store=STORE, indt=INDT, xbar=XBAR, part="all"):
    import concourse.bass as bass
    import concourse.mybir as mybir
    import concourse.tile as tile
    from concourse import bacc
    from concourse.bass import ds, ts
    from concourse.masks import make_identity
    from contextlib import ExitStack

    f32 = mybir.dt.float32
    bf16 = mybir.dt.bfloat16
    f32r = mybir.dt.float32r
    Sin = mybir.ActivationFunctionType.Sin
    mult = mybir.AluOpType.mult
    add = mybir.AluOpType.add
    is_gt = mybir.AluOpType.is_gt

    mmdt = {"f32": f32, "f32r": f32r, "bf16": bf16}[mm]  # matmul operand tiles
    stdt = {"f32": f32, "bf16": bf16}[store]             # out/wn storage
    idt = {"f32": f32, "bf16": bf16}[indt]               # x/w/wlab input dtype
    if xbar:
        assert mmdt is bf16 and stdt is bf16, "xbar transpose needs 2-byte dtypes"

    nc = bacc.Bacc("TRN2", target_bir_lowering=False, debug=False)

    # packed, partition-major layouts -> every DMA is contiguous
    x_ext = nc.dram_tensor("x", [128, NBT, IN], idt, kind="ExternalInput")
    w_ext = nc.dram_tensor("w", [NOG, 128, 4, IN], idt, kind="ExternalInput")
    wl_ext = nc.dram_tensor("wlab", [128, NBT, IN], idt, kind="ExternalInput")
    m_ext = nc.dram_tensor("m", [1, 1], f32, kind="ExternalInput")
    out_ext = nc.dram_tensor("out", [NOG, 128, NBT, 512], stdt,
                             kind="ExternalOutput")
    wn_ext = nc.dram_tensor("wn", [NOG, 128, 4, IN], stdt, kind="ExternalOutput")
    v_ext = nc.dram_tensor("v", [128, NBT], f32, kind="ExternalOutput")
    trig_dram = nc.dram_tensor("trig_scratch", [1, 2], f32)

    with ExitStack() as ctx:
        tc = ctx.enter_context(tile.TileContext(nc))
        singles = ctx.enter_context(tc.tile_pool(name="singles", bufs=1))
        xin = ctx.enter_context(tc.tile_pool(name="xin", bufs=1))
        small = ctx.enter_context(tc.tile_pool(name="small", bufs=8))
        wpool = ctx.enter_context(tc.tile_pool(name="wpool", bufs=4))
        wnpool = ctx.enter_context(tc.tile_pool(name="wnpool", bufs=4))
        wntp = ctx.enter_context(tc.tile_pool(name="wntp", bufs=4))
        outp = ctx.enter_context(tc.tile_pool(name="outp", bufs=4))
        psum_t = ctx.enter_context(tc.tile_pool(name="psum_t", bufs=3, space="PSUM"))
        psum_mm = ctx.enter_context(tc.tile_pool(name="psum_mm", bufs=5, space="PSUM"))

        def emit_body():
            identity = singles.tile([128, 128], idt)
            make_identity(nc, identity)
            if stdt is not idt:
                identity_st = singles.tile([128, 128], stdt)
                make_identity(nc, identity_st)
            else:
                identity_st = identity

            # cos(m), -sin(m) -> broadcast to all partitions via a DRAM bounce
            m_sb = singles.tile([1, 1], f32)
            nc.sync.dma_start(out=m_sb, in_=m_ext[:, :])
            trig = singles.tile([1, 2], f32)
            halfpi = singles.tile([1, 1], f32)
            nc.vector.memset(halfpi, HALF_PI)
            nc.scalar.activation(trig[:, 0:1], m_sb, Sin, scale=-1.0)    # -sin(m)
            nc.scalar.activation(trig[:, 1:2], m_sb, Sin, bias=halfpi)   # cos(m)
            nc.sync.dma_start(out=trig_dram[:, :], in_=trig)
            trig_bc = singles.tile([128, 2], f32)
            nc.sync.dma_start(out=trig_bc, in_=trig_dram[:, :].to_broadcast([128, 2]))
            negsinm = trig_bc[:, 0:1]
            cosm = trig_bc[:, 1:2]

            # x^T, kept resident: [128, KT, batch]
            xT = singles.tile([128, KT, B], mmdt, bufs=2)

            xall = xin.tile([128, NBT, IN], idt, bufs=2)
            nc.sync.dma_start(out=xall, in_=x_ext[:, :, :])
            wlall = xin.tile([128, NBT, IN], idt, bufs=2)
            nc.sync.dma_start(out=wlall, in_=wl_ext[:, :, :])
            vstage = xin.tile([128, NBT], f32, tag="vstage", bufs=2)

            for bt in range(NBT):
                xt = xall[:, bt, :]
                wl = wlall[:, bt, :]
                scr = xin.tile([128, IN], idt, tag="scr", bufs=3)
                ssx = small.tile([128, 1], f32)
                nc.vector.scalar_tensor_tensor(
                    out=scr, in0=xt, scalar=1.0, in1=xt, op0=mult, op1=mult,
                    accum_out=ssx)
                norm = small.tile([128, 1], f32)
                nc.scalar.sqrt(norm, ssx)
                ssw = small.tile([128, 1], f32)
                nc.vector.scalar_tensor_tensor(
                    out=scr, in0=wl, scalar=1.0, in1=wl, op0=mult, op1=mult,
                    accum_out=ssw)
                wnorm = small.tile([128, 1], f32)
                nc.scalar.sqrt(wnorm, ssw)
                dotv = small.tile([128, 1], f32)
                nc.vector.scalar_tensor_tensor(
                    out=scr, in0=xt, scalar=1.0, in1=wl, op0=mult, op1=mult,
                    accum_out=dotv)

                den = small.tile([128, 1], f32)
                normc = small.tile([128, 1], f32)
                nc.vector.tensor_scalar_max(normc, norm, EPS)
                wnormc = small.tile([128, 1], f32)
                nc.vector.tensor_scalar_max(wnormc, wnorm, EPS)
                nc.vector.tensor_mul(den, normc, wnormc)
                rden = small.tile([128, 1], f32)
                nc.vector.reciprocal(rden, den)
                cl = small.tile([128, 1], f32)
                nc.vector.tensor_mul(cl, dotv, rden)

                mask = small.tile([128, 1], f32)
                nc.vector.tensor_scalar(
                    out=mask, in0=cl, scalar1=0.0, scalar2=None, op0=is_gt)
                sine = small.tile([128, 1], f32)
                nc.vector.tensor_mul(sine, cl, cl)
                nc.vector.tensor_scalar(
                    out=sine, in0=sine, scalar1=-1.0, scalar2=1.0, op0=mult, op1=add)
                nc.vector.tensor_scalar_max(sine, sine, 0.0)
                nc.scalar.sqrt(sine, sine)
                phi = small.tile([128, 1], f32)
                nc.vector.tensor_scalar_mul(phi, cl, cosm)
                nc.vector.scalar_tensor_tensor(
                    out=phi, in0=sine, scalar=negsinm, in1=phi, op0=mult, op1=add)
                dv = small.tile([128, 1], f32)
                nc.vector.tensor_sub(dv, phi, cl)
                vv = small.tile([128, 1], f32)
                nc.vector.scalar_tensor_tensor(
                    out=vv, in0=dv, scalar=mask, in1=cl, op0=mult, op1=add)
                nc.vector.tensor_mul(vstage[:, bt:bt + 1], vv, norm)

                pst = psum_t.tile([128, KT, 128], idt, tag="pst")
                for k in range(KT):
                    nc.tensor.transpose(pst[:, k, :], xt[:, ts(k, 128)], identity)
                nc.scalar.copy(out=xT[:, :, ts(bt, 128)], in_=pst)
            nc.sync.dma_start(out=v_ext[:, :], in_=vstage)

            if part == "mm":
                # timing probe: dense matmul stream against one static wnt
                wnt0 = singles.tile([128, KT, 512], mmdt)
                nc.gpsimd.memset(wnt0, 0.0)
                for og in range(NOG):
                    ostage = outp.tile([128, NBT, 512], stdt)
                    for bt in range(NBT):
                        ps = psum_mm.tile([128, 512], f32)
                        for k in range(KT):
                            nc.tensor.matmul(
                                ps, lhsT=xT[:, k, ts(bt, 128)], rhs=wnt0[:, k, :],
                                start=(k == 0), stop=(k == KT - 1))
                        if bt % 2 == 0:
                            nc.scalar.copy(out=ostage[:, bt, :], in_=ps)
                        else:
                            nc.vector.tensor_copy(out=ostage[:, bt, :], in_=ps)
                    nc.sync.dma_start(out=out_ext[og], in_=ostage)
                return

            for og in range(NOG):
                w4 = wpool.tile([128, 4, IN], idt)
                nc.sync.dma_start(out=w4, in_=w_ext[og])
                wn4 = wnpool.tile([128, 4, IN], stdt)
                wnt = wntp.tile([128, KT, 512], mmdt)

                ssw4 = small.tile([128, 4], f32, tag="ssw4")
                for j in range(4):
                    wscr = wpool.tile([128, IN], idt, tag="wscr")
                    nc.vector.scalar_tensor_tensor(
                        out=wscr, in0=w4[:, j, :], scalar=1.0, in1=w4[:, j, :],
                        op0=mult, op1=mult, accum_out=ssw4[:, j:j + 1])
                wnn4 = small.tile([128, 4], f32, tag="wnn4")
                nc.scalar.sqrt(wnn4, ssw4)
                nc.vector.tensor_scalar_max(wnn4, wnn4, EPS)
                winv4 = small.tile([128, 4], f32, tag="winv4")
                nc.vector.reciprocal(winv4, wnn4)

                for j in range(4):
                    if xbar:
                        nc.gpsimd.tensor_scalar_mul(
                            wn4[:, j, :], w4[:, j, :], winv4[:, j:j + 1])
                        nc.sync.dma_start_transpose(
                            out=wnt[:, :, ts(j, 128)], in_=wn4[:, j, :])
                    else:
                        if j % 2 == 0:
                            nc.vector.tensor_scalar_mul(
                                wn4[:, j, :], w4[:, j, :], winv4[:, j:j + 1])
                        else:
                            nc.scalar.mul(
                                wn4[:, j, :], w4[:, j, :], winv4[:, j:j + 1])
                        pst2 = psum_t.tile([128, KT, 128], stdt, tag="pst")
                        for k in range(KT):
                            nc.tensor.transpose(
                                pst2[:, k, :], wn4[:, j, ts(k, 128)], identity_st)
                        if j % 2 == 0:
                            nc.scalar.copy(out=wnt[:, :, ts(j, 128)], in_=pst2)
                        else:
                            nc.vector.tensor_copy(out=wnt[:, :, ts(j, 128)], in_=pst2)
                nc.sync.dma_start(out=wn_ext[og], in_=wn4)
                if part == "prep":
                    continue

                ostage = outp.tile([128, NBT, 512], stdt)
                for bt in range(NBT):
                    ps = psum_mm.tile([128, 512], f32)
                    for k in range(KT):
                        nc.tensor.matmul(
                            ps, lhsT=xT[:, k, ts(bt, 128)], rhs=wnt[:, k, :],
                            start=(k == 0), stop=(k == KT - 1))
                    if bt % 2 == 0:
                        nc.scalar.copy(out=ostage[:, bt, :], in_=ps)
                    else:
                        nc.vector.tensor_copy(out=ostage[:, bt, :], in_=ps)
                nc.sync.dma_start(out=out_ext[og], in_=ostage)

        if reps == 1:
            emit_body()
        else:
            with tc.For_i(0, reps, 1):
                emit_body()

    nc.compile()
    return nc


def _run(nc, in_maps, trace=False):
    from concourse.bass_utils import run_bass_kernel_spmd

    return run_bass_kernel_spmd(
        nc, in_maps, core_ids=list(range(len(in_maps))), trace=trace)


def _pack_bt(a):
    """[B, IN] row-major -> [128, NBT, IN] partition-major."""
    return np.ascontiguousarray(a.reshape(NBT, 128, IN).swapaxes(0, 1))


def _make_in_maps(x, label, weight, m, indt=INDT):
    if indt == "bf16":
        import ml_dtypes

        cast = lambda a: a.astype(ml_dtypes.bfloat16)
    else:
        cast = lambda a: np.ascontiguousarray(a, dtype=np.float32)
    wlab = weight[label]  # [B, IN] host gather
    xp = cast(_pack_bt(x))
    wlp = cast(_pack_bt(wlab))
    in_maps = []
    for c in range(NCORES):
        wpad = np.zeros((PAD, IN), np.float32)
        wpad[:SLAB] = weight[c * SLAB:(c + 1) * SLAB]
        # [PAD, IN] -> [NOG, 128, 4, IN]: row o = og*512 + j*128 + p
        wp = wpad.reshape(NOG, 4, 128, IN).swapaxes(1, 2)
        in_maps.append({"x": xp, "w": cast(wp), "wlab": wlp, "m": m})
    return in_maps


def _assemble(res, label):
    outs, wns = [], []
    for r in res:
        od = np.asarray(r["out"], dtype=np.float32)     # [NOG,128,NBT,512]
        outs.append(od.transpose(2, 1, 0, 3).reshape(B, PAD)[:, :SLAB])
        wd = np.asarray(r["wn"], dtype=np.float32)      # [NOG,128,4,IN]
        wns.append(wd.swapaxes(1, 2).reshape(PAD, IN)[:SLAB])
    out = np.concatenate(outs, axis=1)
    wn = np.concatenate(wns, axis=0)
    v = np.asarray(res[0]["v"])                         # [128, NBT]
    out[np.arange(B), label] = v.T.reshape(B)
    return out, wn


def kernel(**inputs):
    x = np.asarray(inputs["x"], dtype=np.float32)
    label = np.asarray(inputs["label"]).astype(np.int64)
    weight = np.asarray(inputs["weight"], dtype=np.float32)
    m = np.asarray(inputs["m"], dtype=np.float32).reshape(1, 1)

    nc = _build()
    res = _run(nc, _make_in_maps(x, label, weight, m)).results
    return _assemble(res, label)


# revision 28
# speedup vs baseline: 2.9669x; 1.0134x over previous
"""AAMSoftmax (norm-free) loss head on 8 Trainium2 NeuronCores.

Math (reference):
    norm    = ||x_b||                                  [B, 1]
    xn      = x / max(norm, eps); wn = W / max(||W_row||, eps)
    cosine  = xn @ wn.T                                [B, OUT]
    phi     = cos(theta + m) = cosine*cos(m) - sine*sin(m)
    out     = norm * where(onehot(label) & cosine > 0, phi, cosine)
    returns (out, wn)

Key identity: norm * cosine == x @ wn.T exactly (norm >> eps), so the big
[B, OUT] tensor is a single matmul of the *unnormalized* x against the
row-normalized weights; the margin substitution only touches B entries
(one per row, at column label_b). Each core computes its 1/8 slab of
out/wn plus the per-row substituted value v_b = norm_b * (cl>0 ? phi : cl)
(cl = cosine at the label column, computed from host-gathered weight rows).
The host assembles slabs and writes v at the B label positions.

Sharding: W rows (out_features) split 8 ways, 8000 rows/core padded to
8192; x/wlab/m replicated. All device I/O tensors use partition-major
packed layouts (host packs/unpacks) so every DMA is fully contiguous.
"""

import functools

import numpy as np

B, IN, OUT = 1024, 512, 64000
NCORES = 8
SLAB = OUT // NCORES  # 8000
PAD = 8192            # per-core weight rows, padded for uniform 128/512 tiling
EPS = 1e-12
HALF_PI = 1.5707963267948966

KT = IN // 128    # contraction chunks (4)
NBT = B // 128    # batch tiles (8)
NOG = PAD // 512  # out-feature groups (16)

MM = "bf16"     # matmul operand mode: f32 | f32r | bf16
STORE = "bf16"  # out/wn DRAM dtype: f32 | bf16
INDT = "f32"    # x/w/wlab DRAM dtype: f32 | bf16
XBAR = False    # build wn^T via DMA xbar transpose (needs mm=store=bf16)


@functools.lru_cache(maxsize=None)
def _build(reps=1, mm=MM, store=STORE, indt=INDT, xbar=XBAR, part="all"):
    import concourse.bass as bass
    import concourse.mybir as mybir
    import concourse.tile as tile
    from concourse import bacc
    from concourse.bass import ds, ts
    from concourse.masks import make_identity
    from contextlib import ExitStack

    f32 = mybir.dt.float32
    bf16 = mybir.dt.bfloat16
    f32r = mybir.dt.float32r
    Sin = mybir.ActivationFunctionType.Sin
    mult = mybir.AluOpType.mult
    add = mybir.AluOpType.add
    is_gt = mybir.AluOpType.is_gt

    mmdt = {"f32": f32, "f32r": f32r, "bf16": bf16}[mm]  # matmul operand tiles
    stdt = {"f32": f32, "bf16": bf16}[store]             # out/wn storage
    idt = {"f32": f32, "bf16": bf16}[indt]               # x/w/wlab input dtype
    if xbar:
        assert mmdt is bf16 and stdt is bf16, "xbar transpose needs 2-byte dtypes"

    nc = bacc.Bacc("TRN2", target_bir_lowering=False, debug=False)

    # packed, partition-major layouts -> every DMA is contiguous
    x_ext = nc.dram_tensor("x", [128, NBT, IN], idt, kind="ExternalInput")
    w_ext = nc.dram_tensor("w", [NOG, 128, 4, IN], idt, kind="ExternalInput")
    wl_ext = nc.dram_tensor("wlab", [128, NBT, IN], idt, kind="ExternalInput")
    m_ext = nc.dram_tensor("m", [1, 1], f32, kind="ExternalInput")
    out_ext = nc.dram_tensor("out", [NOG, 128, NBT, 512], stdt,
                             kind="ExternalOutput")
    wn_ext = nc.dram_tensor("wn", [NOG, 128, 4, IN], stdt, kind="ExternalOutput")
    v_ext = nc.dram_tensor("v", [128, NBT], f32, kind="ExternalOutput")
    trig_dram = nc.dram_tensor("trig_scratch", [1, 2], f32)

    with ExitStack() as ctx:
        tc = ctx.enter_context(tile.TileContext(nc))
        singles = ctx.enter_context(tc.tile_pool(name="singles", bufs=1))
        xin = ctx.enter_context(tc.tile_pool(name="xin", bufs=1))
        small = ctx.enter_context(tc.tile_pool(name="small", bufs=8))
        wpool = ctx.enter_context(tc.tile_pool(name="wpool", bufs=4))
        wnpool = ctx.enter_context(tc.tile_pool(name="wnpool", bufs=4))
        wntp = ctx.enter_context(tc.tile_pool(name="wntp", bufs=4))
        outp = ctx.enter_context(tc.tile_pool(name="outp", bufs=4))
        psum_t = ctx.enter_context(tc.tile_pool(name="psum_t", bufs=3, space="PSUM"))
        psum_mm = ctx.enter_context(tc.tile_pool(name="psum_mm", bufs=5, space="PSUM"))

        def emit_body():
            identity = singles.tile([128, 128], idt)
            make_identity(nc, identity)
            if stdt is not idt:
                identity_st = singles.tile([128, 128], stdt)
                make_identity(nc, identity_st)
            else:
                identity_st = identity

            # cos(m), -sin(m) -> broadcast to all partitions via a DRAM bounce
            m_sb = singles.tile([1, 1], f32)
            nc.sync.dma_start(out=m_sb, in_=m_ext[:, :])
            trig = singles.tile([1, 2], f32)
            halfpi = singles.tile([1, 1], f32)
            nc.vector.memset(halfpi, HALF_PI)
            nc.scalar.activation(trig[:, 0:1], m_sb, Sin, scale=-1.0)    # -sin(m)
            nc.scalar.activation(trig[:, 1:2], m_sb, Sin, bias=halfpi)   # cos(m)
            nc.sync.dma_start(out=trig_dram[:, :], in_=trig)
            trig_bc = singles.tile([128, 2], f32)
            nc.sync.dma_start(out=trig_bc, in_=trig_dram[:, :].to_broadcast([128, 2]))
            negsinm = trig_bc[:, 0:1]
            cosm = trig_bc[:, 1:2]

            # x^T, kept resident: [128, KT, batch]
            xT = singles.tile([128, KT, B], mmdt, bufs=2)

            xall = xin.tile([128, NBT, IN], idt, bufs=2)
            nc.sync.dma_start(out=xall, in_=x_ext[:, :, :])
            wlall = xin.tile([128, NBT, IN], idt, bufs=2)
            nc.sync.dma_start(out=wlall, in_=wl_ext[:, :, :])
            vstage = xin.tile([128, NBT], f32, tag="vstage", bufs=2)

            # batched per-row stats: one [128, NBT] column per batch tile
            ssx8 = small.tile([128, NBT], f32, tag="ssx8", bufs=2)
            ssw8 = small.tile([128, NBT], f32, tag="ssw8", bufs=2)
            dot8 = small.tile([128, NBT], f32, tag="dot8", bufs=2)
            for bt in range(NBT):
                xt = xall[:, bt, :]
                wl = wlall[:, bt, :]
                scr = xin.tile([128, IN], idt, tag="scr", bufs=3)
                nc.vector.scalar_tensor_tensor(
                    out=scr, in0=xt, scalar=1.0, in1=xt, op0=mult, op1=mult,
                    accum_out=ssx8[:, bt:bt + 1])
                nc.vector.scalar_tensor_tensor(
                    out=scr, in0=wl, scalar=1.0, in1=wl, op0=mult, op1=mult,
                    accum_out=ssw8[:, bt:bt + 1])
                nc.vector.scalar_tensor_tensor(
                    out=scr, in0=xt, scalar=1.0, in1=wl, op0=mult, op1=mult,
                    accum_out=dot8[:, bt:bt + 1])

                pst = psum_t.tile([128, KT, 128], idt, tag="pst")
                for k in range(KT):
                    nc.tensor.transpose(pst[:, k, :], xt[:, ts(k, 128)], identity)
                nc.scalar.copy(out=xT[:, :, ts(bt, 128)], in_=pst)

            norm8 = small.tile([128, NBT], f32, tag="norm8", bufs=2)
            nc.scalar.sqrt(norm8, ssx8)
            wnorm8 = small.tile([128, NBT], f32, tag="wnorm8", bufs=2)
            nc.scalar.sqrt(wnorm8, ssw8)
            den8 = small.tile([128, NBT], f32, tag="den8", bufs=2)
            nc.vector.tensor_scalar_max(den8, norm8, EPS)
            wnc8 = small.tile([128, NBT], f32, tag="wnc8", bufs=2)
            nc.vector.tensor_scalar_max(wnc8, wnorm8, EPS)
            nc.vector.tensor_mul(den8, den8, wnc8)
            nc.vector.reciprocal(den8, den8)
            cl8 = small.tile([128, NBT], f32, tag="cl8", bufs=2)
            nc.vector.tensor_mul(cl8, dot8, den8)

            mask8 = small.tile([128, NBT], f32, tag="mask8", bufs=2)
            nc.vector.tensor_scalar(
                out=mask8, in0=cl8, scalar1=0.0, scalar2=None, op0=is_gt)
            sine8 = small.tile([128, NBT], f32, tag="sine8", bufs=2)
            nc.vector.tensor_mul(sine8, cl8, cl8)
            nc.vector.tensor_scalar(
                out=sine8, in0=sine8, scalar1=-1.0, scalar2=1.0, op0=mult, op1=add)
            nc.vector.tensor_scalar_max(sine8, sine8, 0.0)
            nc.scalar.sqrt(sine8, sine8)
            phi8 = small.tile([128, NBT], f32, tag="phi8", bufs=2)
            nc.vector.tensor_scalar_mul(phi8, cl8, cosm)
            nc.vector.scalar_tensor_tensor(
                out=phi8, in0=sine8, scalar=negsinm, in1=phi8, op0=mult, op1=add)
            nc.vector.tensor_sub(phi8, phi8, cl8)
            nc.vector.tensor_mul(phi8, phi8, mask8)
            nc.vector.tensor_add(phi8, phi8, cl8)
            nc.vector.tensor_mul(vstage, phi8, norm8)
            nc.sync.dma_start(out=v_ext[:, :], in_=vstage)

            if part == "mm":
                # timing probe: dense matmul stream against one static wnt
                wnt0 = singles.tile([128, KT, 512], mmdt)
                nc.gpsimd.memset(wnt0, 0.0)
                for og in range(NOG):
                    ostage = outp.tile([128, NBT, 512], stdt)
                    for bt in range(NBT):
                        ps = psum_mm.tile([128, 512], f32)
                        for k in range(KT):
                            nc.tensor.matmul(
                                ps, lhsT=xT[:, k, ts(bt, 128)], rhs=wnt0[:, k, :],
                                start=(k == 0), stop=(k == KT - 1))
                        if bt % 2 == 0:
                            nc.scalar.copy(out=ostage[:, bt, :], in_=ps)
                        else:
                            nc.vector.tensor_copy(out=ostage[:, bt, :], in_=ps)
                    nc.sync.dma_start(out=out_ext[og], in_=ostage)
                return

            for og in range(NOG):
                w4 = wpool.tile([128, 4, IN], idt)
                nc.sync.dma_start(out=w4, in_=w_ext[og])
                wn4 = wnpool.tile([128, 4, IN], stdt)
                wnt = wntp.tile([128, KT, 512], mmdt)

                ssw4 = small.tile([128, 4], f32, tag="ssw4")
                for j in range(4):
                    wscr = wpool.tile([128, IN], idt, tag="wscr")
                    nc.vector.scalar_tensor_tensor(
                        out=wscr, in0=w4[:, j, :], scalar=1.0, in1=w4[:, j, :],
                        op0=mult, op1=mult, accum_out=ssw4[:, j:j + 1])
                wnn4 = small.tile([128, 4], f32, tag="wnn4")
                nc.scalar.sqrt(wnn4, ssw4)
                nc.vector.tensor_scalar_max(wnn4, wnn4, EPS)
                winv4 = small.tile([128, 4], f32, tag="winv4")
                nc.vector.reciprocal(winv4, wnn4)

                for j in range(4):
                    if xbar:
                        nc.gpsimd.tensor_scalar_mul(
                            wn4[:, j, :], w4[:, j, :], winv4[:, j:j + 1])
                        nc.sync.dma_start_transpose(
                            out=wnt[:, :, ts(j, 128)], in_=wn4[:, j, :])
                    else:
                        if j % 2 == 0:
                            nc.vector.tensor_scalar_mul(
                                wn4[:, j, :], w4[:, j, :], winv4[:, j:j + 1])
                        else:
                            nc.scalar.mul(
                                wn4[:, j, :], w4[:, j, :], winv4[:, j:j + 1])
                        pst2 = psum_t.tile([128, KT, 128], stdt, tag="pst")
                        for k in range(KT):
                            nc.tensor.transpose(
                                pst2[:, k, :], wn4[:, j, ts(k, 128)], identity_st)
                        if j % 2 == 0:
                            nc.scalar.copy(out=wnt[:, :, ts(j, 128)], in_=pst2)
                        else:
                            nc.vector.tensor_copy(out=wnt[:, :, ts(j, 128)], in_=pst2)
                nc.sync.dma_start(out=wn_ext[og], in_=wn4)
                if part == "prep":
                    continue

                ostage = outp.tile([128, NBT, 512], stdt)
                for bt in range(NBT):
                    ps = psum_mm.tile([128, 512], f32)
                    for k in range(KT):
                        nc.tensor.matmul(
                            ps, lhsT=xT[:, k, ts(bt, 128)], rhs=wnt[:, k, :],
                            start=(k == 0), stop=(k == KT - 1))
                    if bt % 2 == 0:
                        nc.scalar.copy(out=ostage[:, bt, :], in_=ps)
                    else:
                        nc.vector.tensor_copy(out=ostage[:, bt, :], in_=ps)
                nc.sync.dma_start(out=out_ext[og], in_=ostage)

        if reps == 1:
            emit_body()
        else:
            with tc.For_i(0, reps, 1):
                emit_body()

    nc.compile()
    return nc


def _run(nc, in_maps, trace=False):
    from concourse.bass_utils import run_bass_kernel_spmd

    return run_bass_kernel_spmd(
        nc, in_maps, core_ids=list(range(len(in_maps))), trace=trace)


def _pack_bt(a):
    """[B, IN] row-major -> [128, NBT, IN] partition-major."""
    return np.ascontiguousarray(a.reshape(NBT, 128, IN).swapaxes(0, 1))


def _make_in_maps(x, label, weight, m, indt=INDT):
    if indt == "bf16":
        import ml_dtypes

        cast = lambda a: a.astype(ml_dtypes.bfloat16)
    else:
        cast = lambda a: np.ascontiguousarray(a, dtype=np.float32)
    wlab = weight[label]  # [B, IN] host gather
    xp = cast(_pack_bt(x))
    wlp = cast(_pack_bt(wlab))
    in_maps = []
    for c in range(NCORES):
        wpad = np.zeros((PAD, IN), np.float32)
        wpad[:SLAB] = weight[c * SLAB:(c + 1) * SLAB]
        # [PAD, IN] -> [NOG, 128, 4, IN]: row o = og*512 + j*128 + p
        wp = wpad.reshape(NOG, 4, 128, IN).swapaxes(1, 2)
        in_maps.append({"x": xp, "w": cast(wp), "wlab": wlp, "m": m})
    return in_maps


def _assemble(res, label):
    outs, wns = [], []
    for r in res:
        od = np.asarray(r["out"], dtype=np.float32)     # [NOG,128,NBT,512]
        outs.append(od.transpose(2, 1, 0, 3).reshape(B, PAD)[:, :SLAB])
        wd = np.asarray(r["wn"], dtype=np.float32)      # [NOG,128,4,IN]
        wns.append(wd.swapaxes(1, 2).reshape(PAD, IN)[:SLAB])
    out = np.concatenate(outs, axis=1)
    wn = np.concatenate(wns, axis=0)
    v = np.asarray(res[0]["v"])                         # [128, NBT]
    out[np.arange(B), label] = v.T.reshape(B)
    return out, wn


def kernel(**inputs):
    x = np.asarray(inputs["x"], dtype=np.float32)
    label = np.asarray(inputs["label"]).astype(np.int64)
    weight = np.asarray(inputs["weight"], dtype=np.float32)
    m = np.asarray(inputs["m"], dtype=np.float32).reshape(1, 1)

    nc = _build()
    res = _run(nc, _make_in_maps(x, label, weight, m)).results
    return _assemble(res, label)


# revision 33
# speedup vs baseline: 3.7301x; 1.2572x over previous
"""AAMSoftmax (norm-free) loss head on 8 Trainium2 NeuronCores.

Math (reference):
    norm    = ||x_b||                                  [B, 1]
    xn      = x / max(norm, eps); wn = W / max(||W_row||, eps)
    cosine  = xn @ wn.T                                [B, OUT]
    phi     = cos(theta + m) = cosine*cos(m) - sine*sin(m)
    out     = norm * where(onehot(label) & cosine > 0, phi, cosine)
    returns (out, wn)

Key identity: norm * cosine == x @ wn.T exactly (norm >> eps), so the big
[B, OUT] tensor is a single matmul of the *unnormalized* x against the
row-normalized weights; the margin substitution only touches B entries
(one per row, at column label_b). Each core computes its 1/8 slab of
out/wn plus the per-row substituted value v_b = norm_b * (cl>0 ? phi : cl)
(cl = cosine at the label column, computed from host-gathered weight rows).
The host assembles slabs and writes v at the B label positions.

Sharding: W rows (out_features) split 8 ways, 8000 rows/core padded to
8192; x/wlab/m replicated. All device I/O tensors use partition-major
packed layouts (host packs/unpacks) so every DMA is fully contiguous.
"""

import functools

import numpy as np

B, IN, OUT = 1024, 512, 64000
NCORES = 8
SLAB = OUT // NCORES  # 8000
PAD = 8192            # per-core weight rows, padded for uniform 128/512 tiling
EPS = 1e-12
HALF_PI = 1.5707963267948966

KT = IN // 128    # contraction chunks (4)
NBT = B // 128    # batch tiles (8)
NOG = PAD // 512  # out-feature groups (16)

MM = "bf16"     # matmul operand mode: f32 | f32r | bf16
STORE = "bf16"  # out/wn DRAM dtype: f32 | bf16
INDT = "f32"    # x/w/wlab DRAM dtype: f32 | bf16
XBAR = False    # build wn^T via DMA xbar transpose (needs mm=store=bf16)


@functools.lru_cache(maxsize=None)
def _build(reps=1, mm=MM, store=STORE, indt=INDT, xbar=XBAR, part="all",
           split=False):
    import concourse.bass as bass
    import concourse.mybir as mybir
    import concourse.tile as tile
    from concourse import bacc
    from concourse.bass import ds, ts
    from concourse.masks import make_identity
    from contextlib import ExitStack

    f32 = mybir.dt.float32
    bf16 = mybir.dt.bfloat16
    f32r = mybir.dt.float32r
    Sin = mybir.ActivationFunctionType.Sin
    mult = mybir.AluOpType.mult
    add = mybir.AluOpType.add
    is_gt = mybir.AluOpType.is_gt

    mmdt = {"f32": f32, "f32r": f32r, "bf16": bf16}[mm]  # matmul operand tiles
    stdt = {"f32": f32, "bf16": bf16}[store]             # out/wn storage
    idt = {"f32": f32, "bf16": bf16}[indt]               # x/w/wlab input dtype
    if xbar:
        assert mmdt is bf16 and stdt is bf16, "xbar transpose needs 2-byte dtypes"

    nc = bacc.Bacc("TRN2", target_bir_lowering=False, debug=False)

    # packed, partition-major layouts -> every DMA is contiguous
    x_ext = nc.dram_tensor("x", [128, NBT, IN], idt, kind="ExternalInput")
    w_ext = nc.dram_tensor("w", [NOG, 128, 4, IN], idt, kind="ExternalInput")
    wl_ext = nc.dram_tensor("wlab", [128, NBT, IN], idt, kind="ExternalInput")
    m_ext = nc.dram_tensor("m", [1, 1], f32, kind="ExternalInput")
    out_ext = nc.dram_tensor("out", [NOG, 128, NBT, 512], stdt,
                             kind="ExternalOutput")
    wn_ext = nc.dram_tensor("wn", [NOG, 128, 4, IN], stdt, kind="ExternalOutput")
    v_ext = nc.dram_tensor("v", [128, NBT], f32, kind="ExternalOutput")
    trig_dram = nc.dram_tensor("trig_scratch", [1, 2], f32)

    with ExitStack() as ctx:
        tc = ctx.enter_context(tile.TileContext(nc))
        singles = ctx.enter_context(tc.tile_pool(name="singles", bufs=1))
        xin = ctx.enter_context(tc.tile_pool(name="xin", bufs=1))
        small = ctx.enter_context(tc.tile_pool(name="small", bufs=8))
        wpool = ctx.enter_context(tc.tile_pool(name="wpool", bufs=4))
        wnpool = ctx.enter_context(tc.tile_pool(name="wnpool", bufs=4))
        wntp = ctx.enter_context(tc.tile_pool(name="wntp", bufs=4))
        outp = ctx.enter_context(tc.tile_pool(name="outp", bufs=4))
        psum_t = ctx.enter_context(tc.tile_pool(name="psum_t", bufs=3, space="PSUM"))
        psum_mm = ctx.enter_context(tc.tile_pool(name="psum_mm", bufs=5, space="PSUM"))

        def emit_body():
            identity = singles.tile([128, 128], idt)
            make_identity(nc, identity)
            if stdt is not idt:
                identity_st = singles.tile([128, 128], stdt)
                make_identity(nc, identity_st)
            else:
                identity_st = identity

            # cos(m), -sin(m) -> broadcast to all partitions via a DRAM bounce
            m_sb = singles.tile([1, 1], f32)
            nc.sync.dma_start(out=m_sb, in_=m_ext[:, :])
            trig = singles.tile([1, 2], f32)
            halfpi = singles.tile([1, 1], f32)
            nc.vector.memset(halfpi, HALF_PI)
            nc.scalar.activation(trig[:, 0:1], m_sb, Sin, scale=-1.0)    # -sin(m)
            nc.scalar.activation(trig[:, 1:2], m_sb, Sin, bias=halfpi)   # cos(m)
            nc.sync.dma_start(out=trig_dram[:, :], in_=trig)
            trig_bc = singles.tile([128, 2], f32)
            nc.sync.dma_start(out=trig_bc, in_=trig_dram[:, :].to_broadcast([128, 2]))
            negsinm = trig_bc[:, 0:1]
            cosm = trig_bc[:, 1:2]

            # x^T, kept resident: [128, KT, batch]
            xT = singles.tile([128, KT, B], mmdt, bufs=2)

            xall = xin.tile([128, NBT, IN], idt, bufs=2)
            nc.sync.dma_start(out=xall, in_=x_ext[:, :, :])
            wlall = xin.tile([128, NBT, IN], idt, bufs=2)
            nc.sync.dma_start(out=wlall, in_=wl_ext[:, :, :])
            vstage = xin.tile([128, NBT], f32, tag="vstage", bufs=2)

            # batched per-row stats: one [128, NBT] column per batch tile
            ssx8 = small.tile([128, NBT], f32, tag="ssx8", bufs=2)
            ssw8 = small.tile([128, NBT], f32, tag="ssw8", bufs=2)
            dot8 = small.tile([128, NBT], f32, tag="dot8", bufs=2)
            for bt in range(NBT):
                xt = xall[:, bt, :]
                wl = wlall[:, bt, :]
                scr = xin.tile([128, IN], idt, tag="scr", bufs=3)
                nc.vector.scalar_tensor_tensor(
                    out=scr, in0=xt, scalar=1.0, in1=xt, op0=mult, op1=mult,
                    accum_out=ssx8[:, bt:bt + 1])
                nc.vector.scalar_tensor_tensor(
                    out=scr, in0=wl, scalar=1.0, in1=wl, op0=mult, op1=mult,
                    accum_out=ssw8[:, bt:bt + 1])
                nc.vector.scalar_tensor_tensor(
                    out=scr, in0=xt, scalar=1.0, in1=wl, op0=mult, op1=mult,
                    accum_out=dot8[:, bt:bt + 1])

                pst = psum_t.tile([128, KT, 128], idt, tag="pst")
                for k in range(KT):
                    nc.tensor.transpose(pst[:, k, :], xt[:, ts(k, 128)], identity)
                nc.scalar.copy(out=xT[:, :, ts(bt, 128)], in_=pst)

            norm8 = small.tile([128, NBT], f32, tag="norm8", bufs=2)
            nc.scalar.sqrt(norm8, ssx8)
            wnorm8 = small.tile([128, NBT], f32, tag="wnorm8", bufs=2)
            nc.scalar.sqrt(wnorm8, ssw8)
            den8 = small.tile([128, NBT], f32, tag="den8", bufs=2)
            nc.vector.tensor_scalar_max(den8, norm8, EPS)
            wnc8 = small.tile([128, NBT], f32, tag="wnc8", bufs=2)
            nc.vector.tensor_scalar_max(wnc8, wnorm8, EPS)
            nc.vector.tensor_mul(den8, den8, wnc8)
            nc.vector.reciprocal(den8, den8)
            cl8 = small.tile([128, NBT], f32, tag="cl8", bufs=2)
            nc.vector.tensor_mul(cl8, dot8, den8)

            mask8 = small.tile([128, NBT], f32, tag="mask8", bufs=2)
            nc.vector.tensor_scalar(
                out=mask8, in0=cl8, scalar1=0.0, scalar2=None, op0=is_gt)
            sine8 = small.tile([128, NBT], f32, tag="sine8", bufs=2)
            nc.vector.tensor_mul(sine8, cl8, cl8)
            nc.vector.tensor_scalar(
                out=sine8, in0=sine8, scalar1=-1.0, scalar2=1.0, op0=mult, op1=add)
            nc.vector.tensor_scalar_max(sine8, sine8, 0.0)
            nc.scalar.sqrt(sine8, sine8)
            phi8 = small.tile([128, NBT], f32, tag="phi8", bufs=2)
            nc.vector.tensor_scalar_mul(phi8, cl8, cosm)
            nc.vector.scalar_tensor_tensor(
                out=phi8, in0=sine8, scalar=negsinm, in1=phi8, op0=mult, op1=add)
            nc.vector.tensor_sub(phi8, phi8, cl8)
            nc.vector.tensor_mul(phi8, phi8, mask8)
            nc.vector.tensor_add(phi8, phi8, cl8)
            nc.vector.tensor_mul(vstage, phi8, norm8)
            nc.sync.dma_start(out=v_ext[:, :], in_=vstage)

            if part == "mm":
                # timing probe: dense matmul stream against one static wnt
                wnt0 = singles.tile([128, KT, 512], mmdt)
                nc.gpsimd.memset(wnt0, 0.0)
                for og in range(NOG):
                    ostage = outp.tile([128, NBT, 512], stdt)
                    for bt in range(NBT):
                        ps = psum_mm.tile([128, 512], f32)
                        for k in range(KT):
                            nc.tensor.matmul(
                                ps, lhsT=xT[:, k, ts(bt, 128)], rhs=wnt0[:, k, :],
                                start=(k == 0), stop=(k == KT - 1))
                        if bt % 2 == 0:
                            nc.scalar.copy(out=ostage[:, bt, :], in_=ps)
                        else:
                            nc.vector.tensor_copy(out=ostage[:, bt, :], in_=ps)
                    nc.sync.dma_start(out=out_ext[og], in_=ostage)
                return

            wnts = {}
            for og in range(NOG):
                w4 = wpool.tile([128, 4, IN], idt)
                nc.sync.dma_start(out=w4, in_=w_ext[og])
                wn4 = wnpool.tile([128, 4, IN], stdt)
                if split:
                    wnt = wntp.tile([128, KT, 512], mmdt, tag=f"wnt{og}", bufs=1)
                    wnts[og] = wnt
                else:
                    wnt = wntp.tile([128, KT, 512], mmdt)

                ssw4 = small.tile([128, 4], f32, tag="ssw4")
                for j in range(4):
                    wscr = wpool.tile([128, IN], idt, tag="wscr")
                    nc.vector.scalar_tensor_tensor(
                        out=wscr, in0=w4[:, j, :], scalar=1.0, in1=w4[:, j, :],
                        op0=mult, op1=mult, accum_out=ssw4[:, j:j + 1])
                wnn4 = small.tile([128, 4], f32, tag="wnn4")
                nc.scalar.sqrt(wnn4, ssw4)
                nc.vector.tensor_scalar_max(wnn4, wnn4, EPS)
                winv4 = small.tile([128, 4], f32, tag="winv4")
                nc.vector.reciprocal(winv4, wnn4)

                for j in range(4):
                    if xbar:
                        nc.gpsimd.tensor_scalar_mul(
                            wn4[:, j, :], w4[:, j, :], winv4[:, j:j + 1])
                        nc.sync.dma_start_transpose(
                            out=wnt[:, :, ts(j, 128)], in_=wn4[:, j, :])
                    else:
                        if j % 2 == 0:
                            nc.vector.tensor_scalar_mul(
                                wn4[:, j, :], w4[:, j, :], winv4[:, j:j + 1])
                        else:
                            nc.scalar.mul(
                                wn4[:, j, :], w4[:, j, :], winv4[:, j:j + 1])
                        pst2 = psum_t.tile([128, KT, 128], stdt, tag="pst")
                        for k in range(KT):
                            nc.tensor.transpose(
                                pst2[:, k, :], wn4[:, j, ts(k, 128)], identity_st)
                        if j % 2 == 0:
                            nc.scalar.copy(out=wnt[:, :, ts(j, 128)], in_=pst2)
                        else:
                            nc.vector.tensor_copy(out=wnt[:, :, ts(j, 128)], in_=pst2)
                nc.sync.dma_start(out=wn_ext[og], in_=wn4)
                if part == "prep":
                    continue
                if split:
                    continue

                ostage = outp.tile([128, NBT, 512], stdt)
                for bt in range(NBT):
                    ps = psum_mm.tile([128, 512], f32)
                    for k in range(KT):
                        nc.tensor.matmul(
                            ps, lhsT=xT[:, k, ts(bt, 128)], rhs=wnt[:, k, :],
                            start=(k == 0), stop=(k == KT - 1))
                    if bt % 2 == 0:
                        nc.scalar.copy(out=ostage[:, bt, :], in_=ps)
                    else:
                        nc.vector.tensor_copy(out=ostage[:, bt, :], in_=ps)
                nc.sync.dma_start(out=out_ext[og], in_=ostage)

            if split and part != "prep":
                for og in range(NOG):
                    wnt = wnts[og]
                    ostage = outp.tile([128, NBT, 512], stdt)
                    for bt in range(NBT):
                        ps = psum_mm.tile([128, 512], f32)
                        for k in range(KT):
                            nc.tensor.matmul(
                                ps, lhsT=xT[:, k, ts(bt, 128)], rhs=wnt[:, k, :],
                                start=(k == 0), stop=(k == KT - 1))
                        if bt % 2 == 0:
                            nc.scalar.copy(out=ostage[:, bt, :], in_=ps)
                        else:
                            nc.vector.tensor_copy(out=ostage[:, bt, :], in_=ps)
                    nc.sync.dma_start(out=out_ext[og], in_=ostage)

        if reps == 1:
            emit_body()
        else:
            with tc.For_i(0, reps, 1):
                emit_body()

    nc.compile()
    return nc


def _run(nc, in_maps, trace=False):
    from concourse.bass_utils import run_bass_kernel_spmd

    return run_bass_kernel_spmd(
        nc, in_maps, core_ids=list(range(len(in_maps))), trace=trace)


def _pack_bt(a):
    """[B, IN] row-major -> [128, NBT, IN] partition-major."""
    return np.ascontiguousarray(a.reshape(NBT, 128, IN).swapaxes(0, 1))


def _make_in_maps(x, label, weight, m, indt=INDT):
    if indt == "bf16":
        import ml_dtypes

        cast = lambda a: a.astype(ml_dtypes.bfloat16)
    else:
        cast = lambda a: np.ascontiguousarray(a, dtype=np.float32)
    wlab = weight[label]  # [B, IN] host gather
    xp = cast(_pack_bt(x))
    wlp = cast(_pack_bt(wlab))
    in_maps = []
    for c in range(NCORES):
        wpad = np.zeros((PAD, IN), np.float32)
        wpad[:SLAB] = weight[c * SLAB:(c + 1) * SLAB]
        # [PAD, IN] -> [NOG, 128, 4, IN]: row o = og*512 + j*128 + p
        wp = wpad.reshape(NOG, 4, 128, IN).swapaxes(1, 2)
        in_maps.append({"x": xp, "w": cast(wp), "wlab": wlp, "m": m})
    return in_maps


def _assemble(res, label):
    outs, wns = [], []
    for r in res:
        od = np.asarray(r["out"], dtype=np.float32)     # [NOG,128,NBT,512]
        outs.append(od.transpose(2, 1, 0, 3).reshape(B, PAD)[:, :SLAB])
        wd = np.asarray(r["wn"], dtype=np.float32)      # [NOG,128,4,IN]
        wns.append(wd.swapaxes(1, 2).reshape(PAD, IN)[:SLAB])
    out = np.concatenate(outs, axis=1)
    wn = np.concatenate(wns, axis=0)
    v = np.asarray(res[0]["v"])                         # [128, NBT]
    out[np.arange(B), label] = v.T.reshape(B)
    return out, wn


def kernel(**inputs):
    import time

    x = np.asarray(inputs["x"], dtype=np.float32)
    label = np.asarray(inputs["label"]).astype(np.int64)
    weight = np.asarray(inputs["weight"], dtype=np.float32)
    m = np.asarray(inputs["m"], dtype=np.float32).reshape(1, 1)

    nc = _build()
    in_maps = _make_in_maps(x, label, weight, m)
    last = None
    for _ in range(3):  # retry transient device-unrecoverable states
        try:
            res = _run(nc, in_maps).results
            return _assemble(res, label)
        except Exception as e:  # noqa: BLE001
            last = e
            time.sleep(5)
    raise last
